# revision 14
# baseline (speedup 1.0000x reference)
"""
Trainium2 Bass kernel for nn_CrossAttention_62027917689453.

Math (per batch b):
    q = rgb @ Wq                       (N, E)
    k = freq @ Wk                      (N, E)
    scores = q @ k.T / sqrt(E)         (N, N)
    attn = softmax(scores, axis=-1)
    attn_out = attn @ freq             (N, D)
    out = concat([rgb, 0.5 * attn_out], axis=-1)   (N, 2D)

(ifreq / Wv are dead inputs in the reference and are ignored.)

Sharding: data-parallel over batch — 8 batches onto 8 NeuronCores, one
independent (N, N) attention slab per core. Full inputs in, full output out.

Per-core kernel layout choices:
  - All matmul operands are fp8e4 (e4m3) and every GEMM runs in
    perf_mode=DoubleRow (two 128-row contraction chunks per instruction,
    ~1.5x the bf16 PE rate at FD=512).  PSUM accumulates fp32.
  - Full-width (N=2048) blocks: each DoubleRow stationary load is reused by
    four FD=512 matmuls (one per 512-wide PSUM bank slice), which cuts
    LDWEIGHTS count vs. 512-wide blocking.
  - Wq/Wk entries are ~N(0, 1/1024) (std 1/32) which lands in e4m3's
    subnormal range, so both are pre-scaled by 32 at cast time; q/k come out
    scaled by 32 each and the combined 1/(32*32*32) is folded into the exp
    scale (1/32768 = scores/sqrt(E)).
  - exp uses bias=-2.0 (softmax is shift-invariant; the denominator sums the
    same shifted weights, so normalization cancels it exactly).  This keeps
    the largest exp well below the e4m3 max (448) even with fp8 noise.
  - All matmuls contract over the partition dim, so activations are needed
    transposed (d on partitions).  rgbT / freqT blocks are produced with PE
    transposes (fp8, 1 cyc/row) against an fp8 identity matrix; fp8
    transposes must write PSUM with element step 2 (walrus requirement).
  - Scores are computed TRANSPOSED: sT[m, n] = sum_e kT[e,m]^T qT[e,n], which
    makes exp(sT) (layout [m, n]) directly usable as the stationary operand of
    the attention-output matmul U[n, d] = sum_m P[m,n]^T freq[m,d] with freq in
    its natural layout — no transposes of the (N, N) attention matrix.
  - Softmax skips max-subtraction (scores are O(5) for this problem's
    distribution).  The denominator avoids per-(tile, chunk) ones-matmuls
    (each would reload the P stationary): VectorE accumulates
    colacc[mp, n] = sum_mt P[mp, mt, n] in bf16, then one tiny FD=1 matmul
    per n-tile (lhsT = colacc columns, rhs = ones) finishes the 128-way
    partition reduction, landing colsum[n] on partition n as needed.
  - Normalization runs on the ACT engine (Copy activation with a
    per-partition scale AP = 0.5/colsum), keeping DVE free.
"""

import numpy as np

import concourse.bass as bass
import concourse.mybir as mybir
import concourse.tile as tile
from concourse.tile import TileContext

from concourse.masks import make_identity

F32 = mybir.dt.float32
BF16 = mybir.dt.bfloat16
F8 = mybir.dt.float8e4
DR = mybir.MatmulPerfMode.DoubleRow

B = 8          # batches == cores
N = 2048       # sequence length (n and m)
D = 1024       # feature dim (d and e)
P = 128        # partitions
NT = N // P    # 16  row chunks
DC = D // P    # 8   feature chunks
SL = 512       # PSUM bank slice width (fp32)
NS = N // SL   # 4   slices across the full n width

W_SCALE = 32.0            # fp8 pre-scale on Wq/Wk (their entries are ~1/32)
EXP_SCALE = 1.0 / (W_SCALE * W_SCALE * 32.0)   # undo 32*32, then /sqrt(E)
EXP_BIAS = -2.0           # shift-invariant headroom below the e4m3 max


def _split_multi_waits(nc: bass.Bass) -> int:
    """The walrus build in this container cannot encode multi-semaphore waits
    on several instruction structs (CTRL Drain, PSEUDO_DMA_DIRECT2D, ...):
    setupSyncWait throws an internal error.  Rewrite every instruction that
    carries more than one wait so the extra waits sit on standalone
    single-wait EventSemaphore instructions immediately before it."""
    n_split = 0
    for f in nc.m.functions:
        for blk in f.blocks:
            insts = blk.instructions
            new: list = []
            changed = False
            for inst in insts:
                si = inst.sync_info
                if si is not None and len(si.on_wait) > 1:
                    waits = list(si.on_wait)
                    for w in waits[:-1]:
                        n_split += 1
                        ev = mybir.InstEventSemaphore(
                            name=f"I-msw-{n_split}",
                            ins=[],
                            outs=[],
                            sync_info=mybir.SyncInfo(on_wait=[w], on_update=[]),
                        )
                        ev.engine = inst.engine
                        new.append(ev)
                    si.on_wait.clear()
                    si.on_wait.append(waits[-1])
                    changed = True
                new.append(inst)
            if changed:
                insts[:] = new
    return n_split


def build_program(split_waits: bool = True) -> bass.Bass:
    nc = bass.Bass()
    rgb = nc.declare_dram_parameter("rgb", [N, D], F32, isOutput=False)
    freq = nc.declare_dram_parameter("freq", [N, D], F32, isOutput=False)
    wq = nc.declare_dram_parameter("Wq", [D, D], F32, isOutput=False)
    wk = nc.declare_dram_parameter("Wk", [D, D], F32, isOutput=False)
    out = nc.declare_dram_parameter("out", [N, 2 * D], F32, isOutput=True)

    with TileContext(nc) as tc:
        with (
            tc.tile_pool(name="statics", bufs=1) as statics,
            tc.tile_pool(name="ld", bufs=6) as ldp,
            tc.tile_pool(name="col", bufs=2) as colp,
            tc.tile_pool(name="outp", bufs=3) as outp,
            tc.tile_pool(name="small", bufs=8) as smallp,
            tc.tile_pool(name="ps", bufs=4, space="PSUM") as psp,
            tc.tile_pool(name="psu", bufs=2, space="PSUM") as psup,
        ):
            ident = statics.tile([P, P], F8, tag="ident")
            make_identity(nc, ident)
            ones_bf = statics.tile([P, 1], BF16, tag="onesbf")
            nc.vector.memset(ones_bf, 1.0)
            exp_bias = statics.tile([P, 1], F32, tag="expb")
            nc.vector.memset(exp_bias, EXP_BIAS)

            wq_f8 = statics.tile([P, DC, D], F8, tag="wq")
            wk_f8 = statics.tile([P, DC, D], F8, tag="wk")
            freq_f8 = statics.tile([P, NT, D], F8, tag="freqf8")
            rgb_f8 = statics.tile([P, NT, D], F8, tag="rgbf8")
            kt_f8 = statics.tile([P, DC, N], F8, tag="kt")
            qt_f8 = statics.tile([P, DC, N], F8, tag="qt")
            p_blk = statics.tile([P, NT, N], F8, tag="pblk")
            colacc = statics.tile([P, N], BF16, tag="colacc")

            # DMA issue order is the critical-path order: the first PE work
            # (freqT transposes) needs all freq chunks; kT needs Wk; the rgb
            # transposes and Wq follow; passthrough writes go last.
            def load_freq(mc):
                t = ldp.tile([P, D], F32, tag="ld")
                nc.sync.dma_start(out=t, in_=freq[mc * P:(mc + 1) * P, :])
                nc.vector.tensor_copy(out=freq_f8[:, mc, :], in_=t)

            def load_wk(dc):
                t2 = ldp.tile([P, D], F32, tag="ld")
                nc.sync.dma_start(out=t2, in_=wk[dc * P:(dc + 1) * P, :])
                nc.vector.tensor_scalar_mul(wk_f8[:, dc, :], t2, W_SCALE)

            rgb_chunks = []

            # rgb/Wq casts run on the otherwise-idle GpSimd engine so the
            # Vector/Scalar queues stay free for PSUM drains during kT.
            def load_rgb(mc):
                t = ldp.tile([P, D], F32, tag="ld")
                nc.sync.dma_start(out=t, in_=rgb[mc * P:(mc + 1) * P, :])
                nc.gpsimd.tensor_copy(out=rgb_f8[:, mc, :], in_=t)
                rgb_chunks.append(t)

            for mc in range(4):
                load_freq(mc)
            for dc in range(DC):
                load_wk(dc)
            for mc in range(4, NT):
                load_freq(mc)
            for mc in range(NT):
                load_rgb(mc)
            for dc in range(DC):
                t = ldp.tile([P, D], F32, tag="ld")
                nc.sync.dma_start(out=t, in_=wq[dc * P:(dc + 1) * P, :])
                nc.gpsimd.tensor_scalar_mul(wq_f8[:, dc, :], t, W_SCALE)

            # rgb passthrough writes issue after the critical-path loads
            for mc, t in enumerate(rgb_chunks):
                nc.sync.dma_start(out=out[mc * P:(mc + 1) * P, 0:D], in_=t)

            # --- PE transposes: srcT[d, m] for all m, one dc row at a time.
            # fp8 transposes must write PSUM with element step 2.
            def emit_tr(src_f8, dst_col, dc):
                for ms in range(NS):
                    ps_t = psp.tile([P, SL, 2], F8, tag="ps")
                    for s in range(SL // P):
                        mc = ms * (SL // P) + s
                        nc.tensor.transpose(
                            ps_t[:, s * P:(s + 1) * P, 0],
                            src_f8[:, mc, dc * P:(dc + 1) * P],
                            ident,
                        )
                    if ms % 2 == 0:
                        nc.vector.tensor_copy(
                            out=dst_col[:, dc, ms * SL:(ms + 1) * SL],
                            in_=ps_t[:, :, 0],
                        )
                    else:
                        nc.scalar.copy(
                            out=dst_col[:, dc, ms * SL:(ms + 1) * SL],
                            in_=ps_t[:, :, 0],
                        )

            # --- projT[e, :] = W[d, e]^T srcT[d, :]: DoubleRow over d pairs,
            # one stationary load per (et, j) shared by 4 FD=512 matmuls.
            def emit_proj(w_f8, src_col, dst, et):
                acc = psup.tile([P, D], F32, tag="psu")
                a2 = psp.tile([P, SL], F32, tag="ps", name=f"pj_{et}_a")
                a3 = psp.tile([P, SL], F32, tag="ps", name=f"pj_{et}_b")
                accs = [acc[:, 0:SL], acc[:, SL:D], a2, a3]
                for j in range(DC // 2):
                    lhs = w_f8[:, 2 * j:2 * j + 2, et * P:(et + 1) * P]
                    for ms in range(NS):
                        nc.tensor.matmul(
                            accs[ms],
                            lhs,
                            src_col[:, 2 * j:2 * j + 2, ms * SL:(ms + 1) * SL],
                            start=(j == 0),
                            stop=(j == DC // 2 - 1),
                            perf_mode=DR,
                        )
                for ms in range(NS):
                    dst_sl = dst[:, et, ms * SL:(ms + 1) * SL]
                    if ms % 2 == 0:
                        nc.scalar.copy(out=dst_sl, in_=accs[ms])
                    else:
                        nc.vector.tensor_copy(out=dst_sl, in_=accs[ms])

            ftall = colp.tile([P, DC, N], F8, tag="col", name="ftall")
            for dc in range(DC):
                emit_tr(freq_f8, ftall, dc)
            for et in range(DC):
                emit_proj(wk_f8, ftall, kt_f8, et)
            rtall = colp.tile([P, DC, N], F8, tag="col", name="rtall")
            for dc in range(DC):
                emit_tr(rgb_f8, rtall, dc)
            for et in range(DC):
                emit_proj(wq_f8, rtall, qt_f8, et)

            # --- scoresT[m, :] -> P = exp(scoresT * EXP_SCALE + EXP_BIAS),
            # then colacc[mp, n] += P[mp, mt, n] on VectorE (bf16).
            for mt in range(NT):
                acc = psup.tile([P, D], F32, tag="psu")
                a2 = psp.tile([P, SL], F32, tag="ps", name=f"sc_{mt}_a")
                a3 = psp.tile([P, SL], F32, tag="ps", name=f"sc_{mt}_b")
                accs = [acc[:, 0:SL], acc[:, SL:D], a2, a3]
                for j in range(DC // 2):
                    lhs = kt_f8[:, 2 * j:2 * j + 2, mt * P:(mt + 1) * P]
                    for ms in range(NS):
                        nc.tensor.matmul(
                            accs[ms],
                            lhs,
                            qt_f8[:, 2 * j:2 * j + 2, ms * SL:(ms + 1) * SL],
                            start=(j == 0),
                            stop=(j == DC // 2 - 1),
                            perf_mode=DR,
                        )
                for ms in range(NS):
                    nc.scalar.activation(
                        out=p_blk[:, mt, ms * SL:(ms + 1) * SL],
                        in_=accs[ms],
                        func=mybir.ActivationFunctionType.Exp,
                        scale=EXP_SCALE,
                        bias=exp_bias,
                    )
                if mt == 0:
                    nc.vector.tensor_copy(out=colacc, in_=p_blk[:, 0, :])
                else:
                    nc.vector.tensor_tensor(
                        out=colacc, in0=colacc, in1=p_blk[:, mt, :],
                        op=mybir.AluOpType.add,
                    )

            # --- U[n, d] = sum_m P[m, n]^T freq[m, d], colsum via one tiny
            # FD=1 matmul per n-tile (128-way partition reduction of colacc),
            # normalize on ACT (Copy with scale AP = 0.5/colsum).
            # Emission order: the colsum matmuls + reciprocals are emitted
            # after two U j-loops so the PE has work while the colacc chain
            # (VectorE) finishes; each normalize trails its U tile by two so
            # the psup WAR wait is always satisfied by then.
            ps_us = [None] * NT
            rc2_all = smallp.tile([P, NT], F32, tag="rc2")

            def emit_u(ntl):
                n0 = ntl * P
                ps_u = psup.tile([P, D], F32, tag="psu")
                ps_us[ntl] = ps_u
                for j in range(NT // 2):
                    lhs = p_blk[:, 2 * j:2 * j + 2, n0:n0 + P]
                    nc.tensor.matmul(
                        ps_u[:, 0:SL], lhs,
                        freq_f8[:, 2 * j:2 * j + 2, 0:SL],
                        start=(j == 0), stop=(j == NT // 2 - 1),
                        perf_mode=DR,
                    )
                    nc.tensor.matmul(
                        ps_u[:, SL:D], lhs,
                        freq_f8[:, 2 * j:2 * j + 2, SL:D],
                        start=(j == 0), stop=(j == NT // 2 - 1),
                        perf_mode=DR,
                    )

            def emit_cs(ntl):
                n0 = ntl * P
                ps_cs = psp.tile([P, 1], F32, tag="ps", name=f"cs_{ntl}")
                nc.tensor.matmul(
                    ps_cs, colacc[:, n0:n0 + P], ones_bf,
                    start=True, stop=True,
                )
                rc = smallp.tile([P, 1], F32, tag="rc")
                nc.vector.reciprocal(rc, ps_cs)
                nc.vector.tensor_scalar_mul(rc2_all[:, ntl:ntl + 1], rc, 0.5)

            def emit_norm(ntl):
                n0 = ntl * P
                ot = outp.tile([P, D], F32, tag="ot")
                nc.scalar.activation(
                    out=ot, in_=ps_us[ntl],
                    func=mybir.ActivationFunctionType.Copy,
                    scale=rc2_all[:, ntl:ntl + 1],
                )
                nc.sync.dma_start(out=out[n0:n0 + P, D:2 * D], in_=ot)

            emit_u(0)
            emit_u(1)
            for ntl in range(NT):
                emit_cs(ntl)
            for ntl in range(2, NT):
                emit_u(ntl)
                emit_norm(ntl - 2)
            emit_norm(NT - 2)
            emit_norm(NT - 1)

    if split_waits:
        _split_multi_waits(nc)
    return nc


_CACHE: dict = {}


def _get_program() -> bass.Bass:
    if "nc" not in _CACHE:
        _CACHE["nc"] = build_program()
    return _CACHE["nc"]


def _run(in_maps, trace=False, **kw):
    from concourse.bass_utils import run_bass_kernel_spmd

    nc = _get_program()
    return run_bass_kernel_spmd(nc, in_maps, list(range(B)), trace=trace, **kw)


def kernel(rgb, freq, ifreq=None, Wq=None, Wk=None, Wv=None, **_unused):
    rgb = np.asarray(rgb, dtype=np.float32)
    freq = np.asarray(freq, dtype=np.float32)
    Wq = np.ascontiguousarray(np.asarray(Wq, dtype=np.float32))
    Wk = np.ascontiguousarray(np.asarray(Wk, dtype=np.float32))
    in_maps = [
        {
            "rgb": np.ascontiguousarray(rgb[c]),
            "freq": np.ascontiguousarray(freq[c]),
            "Wq": Wq,
            "Wk": Wk,
        }
        for c in range(B)
    ]
    res = _run(in_maps, trace=False)
    return np.stack([res.results[c]["out"] for c in range(B)], axis=0)


# revision 17
# speedup vs baseline: 1.1981x; 1.1981x over previous
"""
Trainium2 Bass kernel for nn_CrossAttention_62027917689453.

Math (per batch b):
    q = rgb @ Wq                       (N, E)
    k = freq @ Wk                      (N, E)
    scores = q @ k.T / sqrt(E)         (N, N)
    attn = softmax(scores, axis=-1)
    attn_out = attn @ freq             (N, D)
    out = concat([rgb, 0.5 * attn_out], axis=-1)   (N, 2D)

(ifreq / Wv are dead inputs in the reference and are ignored.)

Sharding: data-parallel over batch — 8 batches onto 8 NeuronCores, one
independent (N, N) attention slab per core. Full inputs in, full output out.

Per-core kernel layout choices:
  - All matmul operands are fp8e4 (e4m3) and every GEMM runs in
    perf_mode=DoubleRow (two 128-row contraction chunks per instruction,
    ~1.5x the bf16 PE rate at FD=512).  PSUM accumulates fp32.
  - Full-width (N=2048) blocks: each DoubleRow stationary load is reused by
    four FD=512 matmuls (one per 512-wide PSUM bank slice), which cuts
    LDWEIGHTS count vs. 512-wide blocking.
  - Wq/Wk entries are ~N(0, 1/1024) (std 1/32) which lands in e4m3's
    subnormal range, so both are pre-scaled by 32 at cast time; q/k come out
    scaled by 32 each and the combined 1/(32*32*32) is folded into the exp
    scale (1/32768 = scores/sqrt(E)).
  - exp uses bias=-2.0 (softmax is shift-invariant; the denominator sums the
    same shifted weights, so normalization cancels it exactly).  This keeps
    the largest exp well below the e4m3 max (448) even with fp8 noise.
  - All matmuls contract over the partition dim, so activations are needed
    transposed (d on partitions).  rgbT / freqT blocks are produced with PE
    transposes (fp8, 1 cyc/row) against an fp8 identity matrix; fp8
    transposes must write PSUM with element step 2 (walrus requirement).
  - Scores are computed TRANSPOSED: sT[m, n] = sum_e kT[e,m]^T qT[e,n], which
    makes exp(sT) (layout [m, n]) directly usable as the stationary operand of
    the attention-output matmul U[n, d] = sum_m P[m,n]^T freq[m,d] with freq in
    its natural layout — no transposes of the (N, N) attention matrix.
  - Softmax skips max-subtraction (scores are O(5) for this problem's
    distribution).  The denominator avoids per-(tile, chunk) ones-matmuls
    (each would reload the P stationary): VectorE accumulates
    colacc[mp, n] = sum_mt P[mp, mt, n] in bf16, then one tiny FD=1 matmul
    per n-tile (lhsT = colacc columns, rhs = ones) finishes the 128-way
    partition reduction, landing colsum[n] on partition n as needed.
  - Normalization runs on the ACT engine (Copy activation with a
    per-partition scale AP = 0.5/colsum), keeping DVE free.
"""

import numpy as np

import concourse.bass as bass
import concourse.mybir as mybir
import concourse.tile as tile
from concourse.tile import TileContext

from concourse.masks import make_identity

F32 = mybir.dt.float32
BF16 = mybir.dt.bfloat16
F8 = mybir.dt.float8e4
DR = mybir.MatmulPerfMode.DoubleRow

B = 8          # batches == cores
N = 2048       # sequence length (n and m)
D = 1024       # feature dim (d and e)
P = 128        # partitions
NT = N // P    # 16  row chunks
DC = D // P    # 8   feature chunks
SL = 512       # PSUM bank slice width (fp32)
NS = N // SL   # 4   slices across the full n width

W_SCALE = 32.0            # fp8 pre-scale on Wq/Wk (their entries are ~1/32)
EXP_SCALE = 1.0 / (W_SCALE * W_SCALE * 32.0)   # undo 32*32, then /sqrt(E)
EXP_BIAS = -2.0           # shift-invariant headroom below the e4m3 max


def _split_multi_waits(nc: bass.Bass) -> int:
    """The walrus build in this container cannot encode multi-semaphore waits
    on several instruction structs (CTRL Drain, PSEUDO_DMA_DIRECT2D, ...):
    setupSyncWait throws an internal error.  Rewrite every instruction that
    carries more than one wait so the extra waits sit on standalone
    single-wait EventSemaphore instructions immediately before it."""
    n_split = 0
    for f in nc.m.functions:
        for blk in f.blocks:
            insts = blk.instructions
            new: list = []
            changed = False
            for inst in insts:
                si = inst.sync_info
                if si is not None and len(si.on_wait) > 1:
                    waits = list(si.on_wait)
                    for w in waits[:-1]:
                        n_split += 1
                        ev = mybir.InstEventSemaphore(
                            name=f"I-msw-{n_split}",
                            ins=[],
                            outs=[],
                            sync_info=mybir.SyncInfo(on_wait=[w], on_update=[]),
                        )
                        ev.engine = inst.engine
                        new.append(ev)
                    si.on_wait.clear()
                    si.on_wait.append(waits[-1])
                    changed = True
                new.append(inst)
            if changed:
                insts[:] = new
    return n_split


def build_program(split_waits: bool = True) -> bass.Bass:
    nc = bass.Bass()
    rgb = nc.declare_dram_parameter("rgb", [N, D], F32, isOutput=False)
    freq = nc.declare_dram_parameter("freq", [N, D], F32, isOutput=False)
    wq = nc.declare_dram_parameter("Wq", [D, D], F32, isOutput=False)
    wk = nc.declare_dram_parameter("Wk", [D, D], F32, isOutput=False)
    out = nc.declare_dram_parameter("out", [N, 2 * D], F32, isOutput=True)

    with TileContext(nc) as tc:
        with (
            tc.tile_pool(name="statics", bufs=1) as statics,
            tc.tile_pool(name="ld", bufs=6) as ldp,
            tc.tile_pool(name="col", bufs=2) as colp,
            tc.tile_pool(name="outp", bufs=3) as outp,
            tc.tile_pool(name="small", bufs=8) as smallp,
            tc.tile_pool(name="ps", bufs=4, space="PSUM") as psp,
            tc.tile_pool(name="psu", bufs=2, space="PSUM") as psup,
        ):
            ident = statics.tile([P, P], F8, tag="ident")
            make_identity(nc, ident)
            ones_bf = statics.tile([P, 1], BF16, tag="onesbf")
            nc.vector.memset(ones_bf, 1.0)
            exp_bias = statics.tile([P, 1], F32, tag="expb")
            nc.vector.memset(exp_bias, EXP_BIAS)

            wq_f8 = statics.tile([P, DC, D], F8, tag="wq")
            wk_f8 = statics.tile([P, DC, D], F8, tag="wk")
            freq_f8 = statics.tile([P, NT, D], F8, tag="freqf8")
            rgb_f8 = statics.tile([P, NT, D], F8, tag="rgbf8")
            kt_f8 = statics.tile([P, DC, N], F8, tag="kt")
            qt_f8 = statics.tile([P, DC, N], F8, tag="qt")
            p_blk = statics.tile([P, NT, N], F8, tag="pblk")
            colacc = statics.tile([P, N], BF16, tag="colacc")

            # DMA issue order is the critical-path order: the first PE work
            # (freqT transposes) needs all freq chunks; kT needs Wk; the rgb
            # transposes and Wq follow; passthrough writes go last.
            def load_freq(mc):
                t = ldp.tile([P, D], F32, tag="ld")
                nc.sync.dma_start(out=t, in_=freq[mc * P:(mc + 1) * P, :])
                nc.vector.tensor_copy(out=freq_f8[:, mc, :], in_=t)

            def load_wk(dc):
                t2 = ldp.tile([P, D], F32, tag="ld")
                nc.sync.dma_start(out=t2, in_=wk[dc * P:(dc + 1) * P, :])
                nc.vector.tensor_scalar_mul(wk_f8[:, dc, :], t2, W_SCALE)

            rgb_chunks = []

            # rgb/Wq DMAs issue in the prologue, but their f8 casts are
            # emitted interleaved into the kT / rgbT phases so they don't
            # head-block the PSUM-drain copies in the engine queues.
            def load_rgb(mc):
                t = ldp.tile([P, D], F32, tag="ld")
                nc.sync.dma_start(out=t, in_=rgb[mc * P:(mc + 1) * P, :])
                rgb_chunks.append(t)

            def cast_rgb(mc):
                nc.vector.tensor_copy(out=rgb_f8[:, mc, :], in_=rgb_chunks[mc])

            wq_chunks = []

            def cast_wq(dc):
                nc.scalar.activation(
                    out=wq_f8[:, dc, :], in_=wq_chunks[dc],
                    func=mybir.ActivationFunctionType.Copy, scale=W_SCALE,
                )

            for mc in range(4):
                load_freq(mc)
            for dc in range(DC):
                load_wk(dc)
            for mc in range(4, NT):
                load_freq(mc)
            for mc in range(NT):
                load_rgb(mc)
            for dc in range(DC):
                t = ldp.tile([P, D], F32, tag="ld")
                nc.sync.dma_start(out=t, in_=wq[dc * P:(dc + 1) * P, :])
                wq_chunks.append(t)

            # rgb passthrough writes issue after the critical-path loads
            for mc, t in enumerate(rgb_chunks):
                nc.sync.dma_start(out=out[mc * P:(mc + 1) * P, 0:D], in_=t)

            # --- PE transposes: srcT[d, m] for all m, one dc row at a time.
            # fp8 transposes must write PSUM with element step 2.
            def emit_tr(src_f8, dst_col, dc):
                for ms in range(NS):
                    ps_t = psp.tile([P, SL, 2], F8, tag="ps")
                    for s in range(SL // P):
                        mc = ms * (SL // P) + s
                        nc.tensor.transpose(
                            ps_t[:, s * P:(s + 1) * P, 0],
                            src_f8[:, mc, dc * P:(dc + 1) * P],
                            ident,
                        )
                    if ms % 2 == 0:
                        nc.vector.tensor_copy(
                            out=dst_col[:, dc, ms * SL:(ms + 1) * SL],
                            in_=ps_t[:, :, 0],
                        )
                    else:
                        nc.scalar.copy(
                            out=dst_col[:, dc, ms * SL:(ms + 1) * SL],
                            in_=ps_t[:, :, 0],
                        )

            # --- projT[e, :] = W[d, e]^T srcT[d, :]: DoubleRow over d pairs,
            # one stationary load per (et, j) shared by 4 FD=512 matmuls.
            def emit_proj(w_f8, src_col, dst, et):
                acc = psup.tile([P, D], F32, tag="psu")
                a2 = psp.tile([P, SL], F32, tag="ps", name=f"pj_{et}_a")
                a3 = psp.tile([P, SL], F32, tag="ps", name=f"pj_{et}_b")
                accs = [acc[:, 0:SL], acc[:, SL:D], a2, a3]
                for j in range(DC // 2):
                    lhs = w_f8[:, 2 * j:2 * j + 2, et * P:(et + 1) * P]
                    for ms in range(NS):
                        nc.tensor.matmul(
                            accs[ms],
                            lhs,
                            src_col[:, 2 * j:2 * j + 2, ms * SL:(ms + 1) * SL],
                            start=(j == 0),
                            stop=(j == DC // 2 - 1),
                            perf_mode=DR,
                        )
                for ms in range(NS):
                    dst_sl = dst[:, et, ms * SL:(ms + 1) * SL]
                    if ms % 2 == 0:
                        nc.scalar.copy(out=dst_sl, in_=accs[ms])
                    else:
                        nc.vector.tensor_copy(out=dst_sl, in_=accs[ms])

            ftall = colp.tile([P, DC, N], F8, tag="col", name="ftall")
            for dc in range(DC):
                emit_tr(freq_f8, ftall, dc)
            for et in range(DC):
                emit_proj(wk_f8, ftall, kt_f8, et)
                cast_rgb(2 * et)
                cast_rgb(2 * et + 1)
            rtall = colp.tile([P, DC, N], F8, tag="col", name="rtall")
            for dc in range(DC):
                emit_tr(rgb_f8, rtall, dc)
                cast_wq(dc)
            for et in range(DC):
                emit_proj(wq_f8, rtall, qt_f8, et)

            # --- scoresT[m, :] -> P = exp(scoresT * EXP_SCALE + EXP_BIAS),
            # then colacc[mp, n] += P[mp, mt, n] on VectorE (bf16).
            for mt in range(NT):
                acc = psup.tile([P, D], F32, tag="psu")
                a2 = psp.tile([P, SL], F32, tag="ps", name=f"sc_{mt}_a")
                a3 = psp.tile([P, SL], F32, tag="ps", name=f"sc_{mt}_b")
                accs = [acc[:, 0:SL], acc[:, SL:D], a2, a3]
                for j in range(DC // 2):
                    lhs = kt_f8[:, 2 * j:2 * j + 2, mt * P:(mt + 1) * P]
                    for ms in range(NS):
                        nc.tensor.matmul(
                            accs[ms],
                            lhs,
                            qt_f8[:, 2 * j:2 * j + 2, ms * SL:(ms + 1) * SL],
                            start=(j == 0),
                            stop=(j == DC // 2 - 1),
                            perf_mode=DR,
                        )
                for ms in range(NS):
                    nc.scalar.activation(
                        out=p_blk[:, mt, ms * SL:(ms + 1) * SL],
                        in_=accs[ms],
                        func=mybir.ActivationFunctionType.Exp,
                        scale=EXP_SCALE,
                        bias=exp_bias,
                    )
                if mt == 0:
                    nc.vector.tensor_copy(out=colacc, in_=p_blk[:, 0, :])
                else:
                    nc.vector.tensor_tensor(
                        out=colacc, in0=colacc, in1=p_blk[:, mt, :],
                        op=mybir.AluOpType.add,
                    )

            # --- U[n, d] = sum_m P[m, n]^T freq[m, d], colsum via one tiny
            # FD=1 matmul per n-tile (128-way partition reduction of colacc),
            # normalize on ACT (Copy with scale AP = 0.5/colsum).
            # Emission order: the colsum matmuls + reciprocals are emitted
            # after two U j-loops so the PE has work while the colacc chain
            # (VectorE) finishes; each normalize trails its U tile by two so
            # the psup WAR wait is always satisfied by then.
            ps_us = [None] * NT
            rc2_all = smallp.tile([P, NT], F32, tag="rc2")

            def emit_u(ntl):
                n0 = ntl * P
                ps_u = psup.tile([P, D], F32, tag="psu")
                ps_us[ntl] = ps_u
                for j in range(NT // 2):
                    lhs = p_blk[:, 2 * j:2 * j + 2, n0:n0 + P]
                    nc.tensor.matmul(
                        ps_u[:, 0:SL], lhs,
                        freq_f8[:, 2 * j:2 * j + 2, 0:SL],
                        start=(j == 0), stop=(j == NT // 2 - 1),
                        perf_mode=DR,
                    )
                    nc.tensor.matmul(
                        ps_u[:, SL:D], lhs,
                        freq_f8[:, 2 * j:2 * j + 2, SL:D],
                        start=(j == 0), stop=(j == NT // 2 - 1),
                        perf_mode=DR,
                    )

            def emit_cs(ntl):
                n0 = ntl * P
                ps_cs = psp.tile([P, 1], F32, tag="ps", name=f"cs_{ntl}")
                nc.tensor.matmul(
                    ps_cs, colacc[:, n0:n0 + P], ones_bf,
                    start=True, stop=True,
                )
                rc = smallp.tile([P, 1], F32, tag="rc")
                nc.vector.reciprocal(rc, ps_cs)
                nc.vector.tensor_scalar_mul(rc2_all[:, ntl:ntl + 1], rc, 0.5)

            def emit_norm(ntl):
                n0 = ntl * P
                ot = outp.tile([P, D], F32, tag="ot")
                nc.scalar.activation(
                    out=ot, in_=ps_us[ntl],
                    func=mybir.ActivationFunctionType.Copy,
                    scale=rc2_all[:, ntl:ntl + 1],
                )
                nc.sync.dma_start(out=out[n0:n0 + P, D:2 * D], in_=ot)

            emit_u(0)
            emit_u(1)
            for ntl in range(NT):
                emit_cs(ntl)
            for ntl in range(2, NT):
                emit_u(ntl)
                emit_norm(ntl - 2)
            emit_norm(NT - 2)
            emit_norm(NT - 1)

    if split_waits:
        _split_multi_waits(nc)
    return nc


_CACHE: dict = {}


def _get_program() -> bass.Bass:
    if "nc" not in _CACHE:
        _CACHE["nc"] = build_program()
    return _CACHE["nc"]


def _run(in_maps, trace=False, **kw):
    from concourse.bass_utils import run_bass_kernel_spmd

    nc = _get_program()
    return run_bass_kernel_spmd(nc, in_maps, list(range(B)), trace=trace, **kw)


def kernel(rgb, freq, ifreq=None, Wq=None, Wk=None, Wv=None, **_unused):
    rgb = np.asarray(rgb, dtype=np.float32)
    freq = np.asarray(freq, dtype=np.float32)
    Wq = np.ascontiguousarray(np.asarray(Wq, dtype=np.float32))
    Wk = np.ascontiguousarray(np.asarray(Wk, dtype=np.float32))
    in_maps = [
        {
            "rgb": np.ascontiguousarray(rgb[c]),
            "freq": np.ascontiguousarray(freq[c]),
            "Wq": Wq,
            "Wk": Wk,
        }
        for c in range(B)
    ]
    res = _run(in_maps, trace=False)
    return np.stack([res.results[c]["out"] for c in range(B)], axis=0)


# revision 21
# speedup vs baseline: 1.4154x; 1.1814x over previous
"""
Trainium2 Bass kernel for nn_CrossAttention_62027917689453.

Math (per batch b):
    q = rgb @ Wq                       (N, E)
    k = freq @ Wk                      (N, E)
    scores = q @ k.T / sqrt(E)         (N, N)
    attn = softmax(scores, axis=-1)
    attn_out = attn @ freq             (N, D)
    out = concat([rgb, 0.5 * attn_out], axis=-1)   (N, 2D)

(ifreq / Wv are dead inputs in the reference and are ignored.)

Sharding: data-parallel over batch — 8 batches onto 8 NeuronCores, one
independent (N, N) attention slab per core. Full inputs in, full output out.

Per-core kernel layout choices:
  - All matmul operands are fp8e4 (e4m3) and every GEMM runs in
    perf_mode=DoubleRow (two 128-row contraction chunks per instruction,
    ~1.5x the bf16 PE rate at FD=512).  PSUM accumulates fp32.
  - Full-width (N=2048) blocks: each DoubleRow stationary load is reused by
    four FD=512 matmuls (one per 512-wide PSUM bank slice), which cuts
    LDWEIGHTS count vs. 512-wide blocking.
  - Wq/Wk entries are ~N(0, 1/1024) (std 1/32) which lands in e4m3's
    subnormal range, so both are pre-scaled by 32 at cast time; q/k come out
    scaled by 32 each and the combined 1/(32*32*32) is folded into the exp
    scale (1/32768 = scores/sqrt(E)).
  - exp uses bias=-2.0 (softmax is shift-invariant; the denominator sums the
    same shifted weights, so normalization cancels it exactly).  This keeps
    the largest exp well below the e4m3 max (448) even with fp8 noise.
  - All matmuls contract over the partition dim, so activations are needed
    transposed (d on partitions).  rgbT / freqT blocks are produced with PE
    transposes (fp8, 1 cyc/row) against an fp8 identity matrix; fp8
    transposes must write PSUM with element step 2 (walrus requirement).
  - Scores are computed TRANSPOSED: sT[m, n] = sum_e kT[e,m]^T qT[e,n], which
    makes exp(sT) (layout [m, n]) directly usable as the stationary operand of
    the attention-output matmul U[n, d] = sum_m P[m,n]^T freq[m,d] with freq in
    its natural layout — no transposes of the (N, N) attention matrix.
  - Softmax skips max-subtraction (scores are O(5) for this problem's
    distribution).  The denominator avoids per-(tile, chunk) ones-matmuls
    (each would reload the P stationary): VectorE accumulates
    colacc[mp, n] = sum_mt P[mp, mt, n] in bf16, then one tiny FD=1 matmul
    per n-tile (lhsT = colacc columns, rhs = ones) finishes the 128-way
    partition reduction, landing colsum[n] on partition n as needed.
  - Normalization runs on the ACT engine (Copy activation with a
    per-partition scale AP = 0.5/colsum), keeping DVE free.
"""

import numpy as np

import concourse.bass as bass
import concourse.mybir as mybir
import concourse.tile as tile
from concourse.tile import TileContext

from concourse.masks import make_identity

F32 = mybir.dt.float32
BF16 = mybir.dt.bfloat16
F8 = mybir.dt.float8e4
DR = mybir.MatmulPerfMode.DoubleRow

B = 8          # batches == cores
N = 2048       # sequence length (n and m)
D = 1024       # feature dim (d and e)
P = 128        # partitions
NT = N // P    # 16  row chunks
DC = D // P    # 8   feature chunks
SL = 512       # PSUM bank slice width (fp32)
NS = N // SL   # 4   slices across the full n width

W_SCALE = 32.0            # fp8 pre-scale on Wq/Wk (their entries are ~1/32)
EXP_SCALE = 1.0 / (W_SCALE * W_SCALE * 32.0)   # undo 32*32, then /sqrt(E)
EXP_BIAS = -2.0           # shift-invariant headroom below the e4m3 max


def _split_multi_waits(nc: bass.Bass) -> int:
    """The walrus build in this container cannot encode multi-semaphore waits
    on several instruction structs (CTRL Drain, PSEUDO_DMA_DIRECT2D, ...):
    setupSyncWait throws an internal error.  Rewrite every instruction that
    carries more than one wait so the extra waits sit on standalone
    single-wait EventSemaphore instructions immediately before it."""
    n_split = 0
    for f in nc.m.functions:
        for blk in f.blocks:
            insts = blk.instructions
            new: list = []
            changed = False
            for inst in insts:
                si = inst.sync_info
                if si is not None and len(si.on_wait) > 1:
                    waits = list(si.on_wait)
                    for w in waits[:-1]:
                        n_split += 1
                        ev = mybir.InstEventSemaphore(
                            name=f"I-msw-{n_split}",
                            ins=[],
                            outs=[],
                            sync_info=mybir.SyncInfo(on_wait=[w], on_update=[]),
                        )
                        ev.engine = inst.engine
                        new.append(ev)
                    si.on_wait.clear()
                    si.on_wait.append(waits[-1])
                    changed = True
                new.append(inst)
            if changed:
                insts[:] = new
    return n_split


def build_program(split_waits: bool = True) -> bass.Bass:
    nc = bass.Bass()
    rgb = nc.declare_dram_parameter("rgb", [N, D], F32, isOutput=False)
    freq = nc.declare_dram_parameter("freq", [N, D], F32, isOutput=False)
    wq = nc.declare_dram_parameter("Wq", [D, D], F32, isOutput=False)
    wk = nc.declare_dram_parameter("Wk", [D, D], F32, isOutput=False)
    out = nc.declare_dram_parameter("out", [N, 2 * D], F32, isOutput=True)

    with TileContext(nc) as tc:
        with (
            tc.tile_pool(name="statics", bufs=1) as statics,
            tc.tile_pool(name="ld", bufs=8) as ldp,
            tc.tile_pool(name="col", bufs=2) as colp,
            tc.tile_pool(name="outp", bufs=3) as outp,
            tc.tile_pool(name="small", bufs=8) as smallp,
            tc.tile_pool(name="ps", bufs=4, space="PSUM") as psp,
            tc.tile_pool(name="psu", bufs=2, space="PSUM") as psup,
        ):
            ident = statics.tile([P, P], F8, tag="ident")
            make_identity(nc, ident)
            ones_bf = statics.tile([P, 1], BF16, tag="onesbf")
            nc.vector.memset(ones_bf, 1.0)
            exp_bias = statics.tile([P, 1], F32, tag="expb")
            nc.vector.memset(exp_bias, EXP_BIAS)

            wq_f8 = statics.tile([P, DC, D], F8, tag="wq")
            wk_f8 = statics.tile([P, DC, D], F8, tag="wk")
            freq_f8 = statics.tile([P, NT, D], F8, tag="freqf8")
            rgb_f8 = statics.tile([P, NT, D], F8, tag="rgbf8")
            kt_f8 = statics.tile([P, DC, N], F8, tag="kt")
            qt_f8 = statics.tile([P, DC, N], F8, tag="qt")
            p_blk = statics.tile([P, NT, N], F8, tag="pblk")
            colacc = statics.tile([P, N], BF16, tag="colacc")

            # DMA issue order is the critical-path order: the first PE work
            # (freqT transposes) needs all freq chunks; kT needs Wk; the rgb
            # transposes and Wq follow; passthrough writes go last.
            def load_freq(mc):
                t = ldp.tile([P, D], F32, tag="ld")
                nc.sync.dma_start(out=t, in_=freq[mc * P:(mc + 1) * P, :])
                nc.vector.tensor_copy(out=freq_f8[:, mc, :], in_=t)

            def load_wk(dc):
                t2 = ldp.tile([P, D], F32, tag="ld")
                nc.sync.dma_start(out=t2, in_=wk[dc * P:(dc + 1) * P, :])
                nc.vector.tensor_scalar_mul(wk_f8[:, dc, :], t2, W_SCALE)

            rgb_chunks = []

            # rgb/Wq DMAs issue in the prologue, but their f8 casts are
            # emitted interleaved into the kT / rgbT phases so they don't
            # head-block the PSUM-drain copies in the engine queues.
            def load_rgb(mc):
                t = ldp.tile([P, D], F32, tag="ld")
                nc.sync.dma_start(out=t, in_=rgb[mc * P:(mc + 1) * P, :])
                rgb_chunks.append(t)

            def cast_rgb(mc):
                nc.vector.tensor_copy(out=rgb_f8[:, mc, :], in_=rgb_chunks[mc])

            wq_chunks = []

            def cast_wq(dc):
                nc.scalar.activation(
                    out=wq_f8[:, dc, :], in_=wq_chunks[dc],
                    func=mybir.ActivationFunctionType.Copy, scale=W_SCALE,
                )

            # all freq casts precede the wk casts on VectorE so the ms-outer
            # freqT transpose stream is never starved by weight casts
            for mc in range(NT):
                load_freq(mc)
            for dc in range(DC):
                load_wk(dc)
            for mc in range(NT):
                load_rgb(mc)
            for dc in range(DC):
                t = ldp.tile([P, D], F32, tag="ld")
                nc.sync.dma_start(out=t, in_=wq[dc * P:(dc + 1) * P, :])
                wq_chunks.append(t)

            # rgb passthrough writes issue after the critical-path loads
            for mc, t in enumerate(rgb_chunks):
                nc.sync.dma_start(out=out[mc * P:(mc + 1) * P, 0:D], in_=t)

            # --- PE transposes: srcT[d, m], emitted ms-OUTER so the first
            # transposes only need the first 4 source-chunk casts (the cast
            # stream pipelines one chunk-group ahead of the PE).
            # fp8 transposes must write PSUM with element step 2.
            def emit_tr_ms(src_f8, dst_col, ms):
                for dc in range(DC):
                    ps_t = psp.tile([P, SL, 2], F8, tag="ps")
                    for s in range(SL // P):
                        mc = ms * (SL // P) + s
                        nc.tensor.transpose(
                            ps_t[:, s * P:(s + 1) * P, 0],
                            src_f8[:, mc, dc * P:(dc + 1) * P],
                            ident,
                        )
                    if dc % 2 == 0:
                        nc.vector.tensor_copy(
                            out=dst_col[:, dc, ms * SL:(ms + 1) * SL],
                            in_=ps_t[:, :, 0],
                        )
                    else:
                        nc.scalar.copy(
                            out=dst_col[:, dc, ms * SL:(ms + 1) * SL],
                            in_=ps_t[:, :, 0],
                        )

            # --- projT[e, :] = W[d, e]^T srcT[d, :]: DoubleRow over d pairs,
            # one stationary load per (et, j) shared by 4 FD=512 matmuls.
            def emit_proj(w_f8, src_col, dst, et):
                acc = psup.tile([P, D], F32, tag="psu")
                a2 = psp.tile([P, SL], F32, tag="ps", name=f"pj_{et}_a")
                a3 = psp.tile([P, SL], F32, tag="ps", name=f"pj_{et}_b")
                accs = [acc[:, 0:SL], acc[:, SL:D], a2, a3]
                for j in range(DC // 2):
                    lhs = w_f8[:, 2 * j:2 * j + 2, et * P:(et + 1) * P]
                    for ms in range(NS):
                        nc.tensor.matmul(
                            accs[ms],
                            lhs,
                            src_col[:, 2 * j:2 * j + 2, ms * SL:(ms + 1) * SL],
                            start=(j == 0),
                            stop=(j == DC // 2 - 1),
                            perf_mode=DR,
                        )
                for ms in range(NS):
                    dst_sl = dst[:, et, ms * SL:(ms + 1) * SL]
                    if ms % 2 == 0:
                        nc.scalar.copy(out=dst_sl, in_=accs[ms])
                    else:
                        nc.vector.tensor_copy(out=dst_sl, in_=accs[ms])

            ftall = colp.tile([P, DC, N], F8, tag="col", name="ftall")
            for ms in range(NS):
                emit_tr_ms(freq_f8, ftall, ms)
            for et in range(DC):
                emit_proj(wk_f8, ftall, kt_f8, et)
                cast_rgb(2 * et)
                cast_rgb(2 * et + 1)
                if et >= 4:
                    cast_wq(2 * (et - 4))
                    cast_wq(2 * (et - 4) + 1)
            rtall = colp.tile([P, DC, N], F8, tag="col", name="rtall")
            for ms in range(NS):
                emit_tr_ms(rgb_f8, rtall, ms)
            for et in range(DC):
                emit_proj(wq_f8, rtall, qt_f8, et)

            # --- scoresT[m, :] -> P = exp(scoresT * EXP_SCALE + EXP_BIAS),
            # then colacc[mp, n] += P[mp, mt, n] on VectorE (bf16).
            for mt in range(NT):
                acc = psup.tile([P, D], F32, tag="psu")
                a2 = psp.tile([P, SL], F32, tag="ps", name=f"sc_{mt}_a")
                a3 = psp.tile([P, SL], F32, tag="ps", name=f"sc_{mt}_b")
                accs = [acc[:, 0:SL], acc[:, SL:D], a2, a3]
                for j in range(DC // 2):
                    lhs = kt_f8[:, 2 * j:2 * j + 2, mt * P:(mt + 1) * P]
                    for ms in range(NS):
                        nc.tensor.matmul(
                            accs[ms],
                            lhs,
                            qt_f8[:, 2 * j:2 * j + 2, ms * SL:(ms + 1) * SL],
                            start=(j == 0),
                            stop=(j == DC // 2 - 1),
                            perf_mode=DR,
                        )
                for ms in range(NS):
                    nc.scalar.activation(
                        out=p_blk[:, mt, ms * SL:(ms + 1) * SL],
                        in_=accs[ms],
                        func=mybir.ActivationFunctionType.Exp,
                        scale=EXP_SCALE,
                        bias=exp_bias,
                    )
                if mt == 0:
                    nc.vector.tensor_copy(out=colacc, in_=p_blk[:, 0, :])
                else:
                    nc.vector.tensor_tensor(
                        out=colacc, in0=colacc, in1=p_blk[:, mt, :],
                        op=mybir.AluOpType.add,
                    )

            # --- U[n, d] = sum_m P[m, n]^T freq[m, d], colsum via one tiny
            # FD=1 matmul per n-tile (128-way partition reduction of colacc),
            # normalize on ACT (Copy with scale AP = 0.5/colsum).
            # Emission order: the colsum matmuls + reciprocals are emitted
            # after two U j-loops so the PE has work while the colacc chain
            # (VectorE) finishes; each normalize trails its U tile by two so
            # the psup WAR wait is always satisfied by then.
            ps_us = [None] * NT
            rc2_all = smallp.tile([P, NT], F32, tag="rc2")

            def emit_u(ntl):
                n0 = ntl * P
                ps_u = psup.tile([P, D], F32, tag="psu")
                ps_us[ntl] = ps_u
                for j in range(NT // 2):
                    lhs = p_blk[:, 2 * j:2 * j + 2, n0:n0 + P]
                    nc.tensor.matmul(
                        ps_u[:, 0:SL], lhs,
                        freq_f8[:, 2 * j:2 * j + 2, 0:SL],
                        start=(j == 0), stop=(j == NT // 2 - 1),
                        perf_mode=DR,
                    )
                    nc.tensor.matmul(
                        ps_u[:, SL:D], lhs,
                        freq_f8[:, 2 * j:2 * j + 2, SL:D],
                        start=(j == 0), stop=(j == NT // 2 - 1),
                        perf_mode=DR,
                    )

            def emit_cs(ntl):
                n0 = ntl * P
                ps_cs = psp.tile([P, 1], F32, tag="ps", name=f"cs_{ntl}")
                nc.tensor.matmul(
                    ps_cs, colacc[:, n0:n0 + P], ones_bf,
                    start=True, stop=True,
                )
                rc = smallp.tile([P, 1], F32, tag="rc")
                nc.vector.reciprocal(rc, ps_cs)
                nc.vector.tensor_scalar_mul(rc2_all[:, ntl:ntl + 1], rc, 0.5)

            def emit_norm(ntl):
                n0 = ntl * P
                ot = outp.tile([P, D], F32, tag="ot")
                nc.scalar.activation(
                    out=ot, in_=ps_us[ntl],
                    func=mybir.ActivationFunctionType.Copy,
                    scale=rc2_all[:, ntl:ntl + 1],
                )
                nc.sync.dma_start(out=out[n0:n0 + P, D:2 * D], in_=ot)

            emit_u(0)
            emit_u(1)
            for ntl in range(NT):
                emit_cs(ntl)
            for ntl in range(2, NT):
                emit_u(ntl)
                emit_norm(ntl - 2)
            emit_norm(NT - 2)
            emit_norm(NT - 1)

    if split_waits:
        _split_multi_waits(nc)
    return nc


_CACHE: dict = {}


def _get_program() -> bass.Bass:
    if "nc" not in _CACHE:
        _CACHE["nc"] = build_program()
    return _CACHE["nc"]


def _run(in_maps, trace=False, **kw):
    from concourse.bass_utils import run_bass_kernel_spmd

    nc = _get_program()
    return run_bass_kernel_spmd(nc, in_maps, list(range(B)), trace=trace, **kw)


def kernel(rgb, freq, ifreq=None, Wq=None, Wk=None, Wv=None, **_unused):
    rgb = np.asarray(rgb, dtype=np.float32)
    freq = np.asarray(freq, dtype=np.float32)
    Wq = np.ascontiguousarray(np.asarray(Wq, dtype=np.float32))
    Wk = np.ascontiguousarray(np.asarray(Wk, dtype=np.float32))
    in_maps = [
        {
            "rgb": np.ascontiguousarray(rgb[c]),
            "freq": np.ascontiguousarray(freq[c]),
            "Wq": Wq,
            "Wk": Wk,
        }
        for c in range(B)
    ]
    res = _run(in_maps, trace=False)
    return np.stack([res.results[c]["out"] for c in range(B)], axis=0)
